# revision 31
# baseline (speedup 1.0000x reference)
"""Trainium2 Bass kernel for 16-head causal MultiHeadAttention.

Problem: x [4, 2048, 1024], 16 heads of dim 64, causal softmax attention,
output projection Wo [1024, 1024] + bo.

Sharding over 8 NeuronCores: core c handles batch b = c // 2 and head-group
g = c % 2 (8 heads each).  Each core computes its 8 heads' Q/K/V projections,
causal attention, and a partial output projection against its row-slice of
Wo.  The two cores of a batch return partial [D, S] outputs that the host
sums, transposes, and biases.

Schedule: the attention inner loop is software-pipelined per chunk (one
128-row t-tile, both heads of a pair side by side): scores matmuls ->
merged causally-trimmed exp (ACT) -> lag-1 AV matmuls, with a filler queue
of projection / V-transpose / output-projection work units drained between
chunks so the PE never idles while ACT runs exp.  Work units are j-ordered
so pair p's attention at s-block j only needs units (p, <=j), letting
pair-0 projections interleave with pair-0 attention (tiny prologue).

Score matmuls contract over dk=64, so the two heads' matmuls run
concurrently in 64x128 row-tiled PE mode (tile positions inferred from
the operands' base partitions); they are issued in batches of two chunks
(4 matmuls) per 64-mode stretch so the 128<->64 mode-switch drain is paid
once per batch.  A junk-transpose warm-up keeps the PE clock ramped
through the initial input-DMA wait; each j-block's softmax normalization
releases the single pa accumulator via one fast ACT copy, with
reciprocal+rescale running off the SBUF copy (reciprocal_approx_fast must
read partition offset 0).  Weights are pre-rearranged host-side so each
pair's projection weights load as plain 2KB-per-partition-line DMAs, and
the partial output is returned as f16 to halve the output-DMA tail.

PSUM (8 banks): sc [128,1024]x2 (scores, both heads) + pa [128,1024]x1
(AV accum + softmax denominators via ones-columns of V') + ps [128,512]x2
(projection groups / V-transposes / output projection / warm-up).
"""

import sys
from collections import deque

for _p in ("/opt/trn_rl_repo", "/root/.axon_site/_ro/trn_rl_repo"):
    if _p not in sys.path:
        sys.path.insert(0, _p)

import numpy as np

import concourse.bacc as bacc
import concourse.mybir as mybir
from concourse import bass_utils
from concourse.masks import make_identity, make_upper_triangular
from concourse.tile import TileContext

P = 128
S = 2048  # sequence length
D = 1024  # hidden size
H = 16  # total heads
DK = 64  # head dim
B = 4  # batch
NCORES = 8
HPC = 8  # heads per core
NPAIR = HPC // 2  # head pairs per core
SB = 512  # s-block width
NSB = S // SB  # 4
TT = S // P  # 16 t-tiles
DT = D // P  # 8 d-tiles

F32 = mybir.dt.float32
F16 = mybir.dt.float16
AF = mybir.ActivationFunctionType
MUL = mybir.AluOpType.mult


def build_nc(debug=False):
    nc = bacc.Bacc()
    xT = nc.dram_tensor("xT", [D, S], F16, kind="ExternalInput")
    # weights pre-rearranged host-side: [r, p*1024 + d*128 + c] =
    # W[d*128 + r, p*128 + c], so each pair loads as one plain
    # [128, 1024] DMA with 2KB partition lines
    wq = nc.dram_tensor("wq", [P, NPAIR * DT * P], F16, kind="ExternalInput")
    wk = nc.dram_tensor("wk", [P, NPAIR * DT * P], F16, kind="ExternalInput")
    wv = nc.dram_tensor("wv", [P, NPAIR * DT * P], F16, kind="ExternalInput")
    wo_t = nc.dram_tensor("wo_t", [HPC * DK, D], F16, kind="ExternalInput")
    bq = nc.dram_tensor("bq", [P, NPAIR], F32, kind="ExternalInput")
    bk = nc.dram_tensor("bk", [P, NPAIR], F32, kind="ExternalInput")
    bv = nc.dram_tensor("bv", [P, NPAIR], F32, kind="ExternalInput")
    out = nc.dram_tensor("out_part", [D, S], F16, kind="ExternalOutput")

    with TileContext(nc) as tc:
        from contextlib import ExitStack

        with ExitStack() as ctx:
            pool = lambda *a, **k: ctx.enter_context(tc.tile_pool(*a, **k))
            xt_pool = pool(name="xt", bufs=DT)
            wgt_pool = pool(name="wgt", bufs=6)
            wo_pool = pool(name="wo", bufs=NPAIR)
            qt_pool = pool(name="qt", bufs=2)
            kt_pool = pool(name="kt", bufs=2)
            vp_pool = pool(name="vp", bufs=2)
            vst_pool = pool(name="vst", bufs=2)
            wt_pool = pool(name="wt", bufs=4)
            ot_pool = pool(name="ot", bufs=NPAIR)
            den_pool = pool(name="den", bufs=2)
            rcs_pool = pool(name="rcs", bufs=2)
            ost_pool = pool(name="ost", bufs=3)
            const_pool = pool(name="const", bufs=1)
            ps_sc = pool(name="ps_sc", bufs=2, space="PSUM")
            ps_pa = pool(name="ps_pa", bufs=1, space="PSUM")
            ps_ps = pool(name="ps_ps", bufs=2, space="PSUM")

            # --- constants ---
            ident = const_pool.tile([P, P], F16)
            make_identity(nc, ident[:])
            # mask2[r, g*128+c] = 1 if c >= r else 0, for both head groups
            mask2 = const_pool.tile([P, 2 * P], F16)
            make_upper_triangular(nc, mask2[:, 0:P], val=1.0, diag=True)
            make_upper_triangular(nc, mask2[:, P : 2 * P], val=1.0, diag=True)
            bq_t = const_pool.tile([P, NPAIR], F32)
            nc.sync.dma_start(bq_t[:], bq[:])
            bk_t = const_pool.tile([P, NPAIR], F32)
            nc.sync.dma_start(bk_t[:], bk[:])
            bv_t = const_pool.tile([P, NPAIR], F32)
            nc.sync.dma_start(bv_t[:], bv[:])

            # --- resident inputs ---
            def load_wgt(srcw, p, name, splits=2):
                t = wgt_pool.tile([P, DT * P], F16, tag="wgt", name=name)
                h = DT * P // splits
                for u in range(splits):
                    nc.sync.dma_start(
                        t[:, u * h : (u + 1) * h],
                        srcw[:, p * DT * P + u * h : p * DT * P + (u + 1) * h],
                    )
                return t

            # PE warm-up: junk transposes during the initial input DMA so
            # the tensor engine is at full clock when real matmuls arrive
            for _ in range(25):
                wpt = ps_ps.tile([P, SB], F16, tag="ps", name="warm")
                for u in range(SB // P):
                    nc.tensor.transpose(
                        wpt[:, u * P : (u + 1) * P], ident[:], ident[:]
                    )

            wtiles = {}  # (p, nm) -> wgt tile
            wtiles[(0, "q")] = load_wgt(wq, 0, "wq0", splits=4)
            wtiles[(0, "k")] = load_wgt(wk, 0, "wk0", splits=4)
            wtiles[(0, "v")] = load_wgt(wv, 0, "wv0", splits=4)
            # j-sliced xT load: s-block 0 lands first (one small DMA per
            # d-tile across parallel queues) so pair-0's j=0 projection
            # chains start ~6us in instead of waiting for the full 4MB
            xt = [
                xt_pool.tile([P, S], F16, tag="xt", name=f"xt{d}")
                for d in range(DT)
            ]
            for d in range(DT):
                nc.sync.dma_start(
                    xt[d][:, 0:SB], xT[d * P : (d + 1) * P, 0:SB]
                )
            for d in range(DT):
                nc.sync.dma_start(
                    xt[d][:, SB : 2 * SB],
                    xT[d * P : (d + 1) * P, SB : 2 * SB],
                )
            for d in range(DT):
                nc.sync.dma_start(
                    xt[d][:, 2 * SB : S], xT[d * P : (d + 1) * P, 2 * SB : S]
                )

            # persistent V' tiles, one per in-flight pair; layout
            # [t, g*2048 + tg*128 + (0:64 V | 64:128 ones)]; the ones
            # halves are written once and survive slot reuse.
            vp_slots = []
            for s in range(2):
                v = vp_pool.tile([P, 2 * TT * P], F16, tag=f"vp{s}", bufs=1,
                                 name=f"vp{s}")
                nc.vector.memset(
                    v[:].rearrange("r (g tg c) -> r g tg c", g=2, tg=TT)[
                        :, :, :, DK:P
                    ],
                    1.0,
                )
                vp_slots.append(v)

            qt_tiles = {}
            kt_tiles = {}
            ot_tiles = []

            # ---- work units (filler between attention chunks) ----
            filler = deque()  # (key, closure)

            reserve = [0]  # units held back for the epilogue

            def drain(n):
                if len(filler) > 12:
                    n += 1
                while n > 0 and len(filler) > reserve[0]:
                    filler.popleft()[1]()
                    n -= 1

            def drain_until(key):
                while any(k <= key for k, _ in filler):
                    filler.popleft()[1]()

            def push_proj_unit(p, nm, j, dest_ap, bias_t):
                def unit():
                    ps = ps_ps.tile([P, SB], F32, tag="ps", name="ps_p")
                    w = wtiles[(p, nm)]
                    for d in range(DT):
                        nc.tensor.matmul(
                            ps[:],
                            w[:, d * P : (d + 1) * P],
                            xt[d][:, j * SB : (j + 1) * SB],
                            start=(d == 0),
                            stop=(d == DT - 1),
                        )
                    nc.vector.tensor_scalar_add(
                        dest_ap, ps[:], bias_t[:, p : p + 1],
                    )

                filler.append(((p, j), unit))

            def push_vtrans_unit(p, j, vst):
                def unit():
                    pt = ps_ps.tile([P, SB], F16, tag="ps", name="pt")
                    for u in range(SB // P):
                        nc.tensor.transpose(
                            pt[:, u * P : (u + 1) * P],
                            vst[:, u * P : (u + 1) * P],
                            ident[:],
                        )
                    # scatter [t, (u, g, c)] -> vp[t, (g, 4j+u, c)]
                    vpt = vp_slots[p % 2]
                    dst = vpt[:].rearrange(
                        "r (g tg c) -> r g tg c", g=2, tg=TT
                    )[:, :, 4 * j : 4 * j + 4, 0:DK]
                    src = pt[:].rearrange(
                        "r (u g c) -> r g u c", u=SB // P, g=2
                    )
                    nc.vector.tensor_copy(dst, src)

                filler.append(((p, j), unit))

            def push_pair_units(p):
                qt = qt_pool.tile([P, S], F16, tag="qt", name=f"qt{p}")
                kt = kt_pool.tile([P, S], F16, tag="kt", name=f"kt{p}")
                qt_tiles[p] = qt
                kt_tiles[p] = kt
                if p > 0:
                    for nm, srcw in (("q", wq), ("k", wk), ("v", wv)):
                        wtiles[(p, nm)] = load_wgt(srcw, p, f"w{nm}{p}")
                # vt-j is pushed one unit after its v-j so the transposes
                # don't immediately wait on the DVE bias-copy of vst
                pend_vt = None
                for j in range(NSB):
                    push_proj_unit(p, "q", j, qt[:, j * SB : (j + 1) * SB],
                                   bq_t)
                    if pend_vt is not None:
                        push_vtrans_unit(p, pend_vt[0], pend_vt[1])
                    push_proj_unit(p, "k", j, kt[:, j * SB : (j + 1) * SB],
                                   bk_t)
                    vst = vst_pool.tile([P, SB], F16, tag="vst", name="vst")
                    push_proj_unit(p, "v", j, vst[:], bv_t)
                    pend_vt = (j, vst)
                push_vtrans_unit(p, pend_vt[0], pend_vt[1])

            def push_oproj_unit(m, j, wo_tiles):
                def unit():
                    ps = ps_ps.tile([P, SB], F32, tag="ps", name="ps_o")
                    for p in range(NPAIR):
                        nc.tensor.matmul(
                            ps[:],
                            wo_tiles[p][:, m * P : (m + 1) * P],
                            ot_tiles[p][:, j * SB : (j + 1) * SB],
                            start=(p == 0),
                            stop=(p == NPAIR - 1),
                        )
                    st = ost_pool.tile([P, SB], F16, tag="ost", name="ost")
                    nc.vector.tensor_copy(st[:], ps[:])
                    # two half-DMAs on separate queues shorten the final
                    # output tail
                    hw_ = SB // 2
                    for u in range(2):
                        nc.sync.dma_start(
                            out[m * P : (m + 1) * P,
                                j * SB + u * hw_ : j * SB + (u + 1) * hw_],
                            st[:, u * hw_ : (u + 1) * hw_],
                        )

                filler.append(((NPAIR, j, m), unit))

            # ---- main schedule ----
            push_pair_units(0)
            wo_tiles = []
            for p in range(NPAIR):
                qt = qt_tiles[p]
                kt = kt_tiles[p]
                vpt = vp_slots[p % 2]
                ot = ot_pool.tile([P, S], F16, tag="ot", name=f"ot{p}")
                ot_tiles.append(ot)
                if p + 1 < NPAIR:
                    push_pair_units(p + 1)
                else:
                    # wo loads land during pair-3 attention; hold back a few
                    # filler units so the epilogue has ready PE work to
                    # overlap the final rescale chain
                    reserve[0] = 6
                    for pp in range(NPAIR):
                        t = wo_pool.tile([P, D], F16, tag="wo", name=f"wo{pp}")
                        nc.sync.dma_start(t[:], wo_t[pp * P : (pp + 1) * P, :])
                        wo_tiles.append(t)

                for j in range(NSB):
                    drain_until((p, j))
                    nt = 4 * j + 4
                    pa = ps_pa.tile([P, 2 * SB], F32, tag="pa", name="pa")
                    pend = []

                    def issue_av(item):
                        i, wt, c0 = item
                        for g in range(2):
                            nc.tensor.matmul(
                                pa[:, g * SB + c0 : (g + 1) * SB],
                                vpt[:, g * TT * P + i * P : g * TT * P + (i + 1) * P],
                                wt[:, g * SB + c0 : (g + 1) * SB],
                                start=(i == 0),
                                stop=(i == nt - 1),
                            )

                    # two chunks per batch: the four 64-contraction score
                    # matmuls run in one 64x128 row-tiled PE stretch (heads
                    # at tile rows 0/64 execute concurrently), paying the
                    # 128<->64 mode-switch drain once per batch.
                    for i2 in range(0, nt, 2):
                        batch = []
                        for i in (i2, i2 + 1):
                            r = i - 4 * j
                            c0 = P * max(r, 0)
                            sc = ps_sc.tile([P, 2 * SB], F32, tag="sc",
                                            name="sc")
                            for g in range(2):
                                nc.tensor.matmul(
                                    sc[:, g * SB + c0 : (g + 1) * SB],
                                    kt[g * DK : (g + 1) * DK,
                                       i * P : (i + 1) * P],
                                    qt[g * DK : (g + 1) * DK,
                                       j * SB + c0 : (j + 1) * SB],
                                    start=True,
                                    stop=True,
                                )
                            batch.append((i, sc, c0))
                        new_pend = []
                        for i, sc, c0 in batch:
                            r = i - 4 * j
                            wt = wt_pool.tile([P, 2 * SB], F16, tag="wt",
                                              name="wt")
                            if r >= 0:
                                scv = sc[:].rearrange(
                                    "r (g w) -> r g w", g=2
                                )[:, :, c0:]
                                wtv = wt[:].rearrange(
                                    "r (g w) -> r g w", g=2
                                )[:, :, c0:]
                            else:
                                scv = sc[:]
                                wtv = wt[:]
                            nc.scalar.activation(wtv, scv, AF.Exp,
                                                 scale=0.125)
                            if r >= 0:
                                wmv = wt[:].rearrange(
                                    "r (g w) -> r g w", g=2
                                )[:, :, c0 : c0 + P]
                                nc.vector.tensor_tensor(
                                    wmv,
                                    wmv,
                                    mask2[:].rearrange(
                                        "r (g w) -> r g w", g=2
                                    ),
                                    MUL,
                                )
                            new_pend.append((i, wt, c0))
                        drain(1)
                        for item in pend:
                            issue_av(item)
                        pend = new_pend
                        drain(1)
                    for item in pend:
                        issue_av(item)
                    # normalize: ones-columns of V' put the softmax
                    # denominator in pa rows 64:127 (replicated).  One fast
                    # ACT copy releases the single pa buffer; recip + muls
                    # then run off the SBUF copy without blocking the PE.
                    pac = den_pool.tile([P, 2 * SB], F32, tag="den",
                                        name="pac")
                    nc.scalar.copy(pac[:], pa[:])
                    den = den_pool.tile([DK, 2 * SB], F32, tag="dend",
                                        name="den")
                    nc.vector.tensor_copy(den[:], pac[DK:P, :])
                    rcs = rcs_pool.tile([DK, 2 * SB], F32, tag="rcs",
                                        name="rcs")
                    nc.vector.reciprocal_approx_fast(rcs[:], den[:])
                    for g in range(2):
                        nc.vector.tensor_tensor(
                            ot[g * DK : (g + 1) * DK, j * SB : (j + 1) * SB],
                            pac[0:DK, g * SB : (g + 1) * SB],
                            rcs[:, g * SB : (g + 1) * SB],
                            MUL,
                        )
                    if p == NPAIR - 1:
                        for m in range(DT):
                            push_oproj_unit(m, j, wo_tiles)
                    drain(1)

            # epilogue: drain remaining output-projection units
            while filler:
                filler.popleft()[1]()

    nc.compile()
    return nc


_NC_CACHE = None


def _get_nc():
    global _NC_CACHE
    if _NC_CACHE is None:
        _NC_CACHE = build_nc()
    return _NC_CACHE


def _core_inputs(x, Wq, bq, Wk, bk, Wv, bv, Wo, c):
    b, g = c // 2, c % 2
    heads = range(g * HPC, (g + 1) * HPC)
    xT = np.ascontiguousarray(x[b].T, dtype=np.float16)
    def warr(W):
        w = np.concatenate([W[h] for h in heads], axis=1)  # [D, 512]
        # [r, p*1024 + d*128 + c] = w[d*128 + r, p*128 + c]
        blocks = [
            w[:, p * P : (p + 1) * P]
            .reshape(DT, P, P)
            .transpose(1, 0, 2)
            .reshape(P, DT * P)
            for p in range(NPAIR)
        ]
        return np.ascontiguousarray(
            np.concatenate(blocks, axis=1), dtype=np.float16
        )

    wq_c = warr(Wq)
    wk_c = warr(Wk)
    wv_c = warr(Wv)
    bq_c = np.ascontiguousarray(
        np.concatenate([bq[h] for h in heads]).reshape(NPAIR, P).T, dtype=np.float32
    )
    bk_c = np.ascontiguousarray(
        np.concatenate([bk[h] for h in heads]).reshape(NPAIR, P).T, dtype=np.float32
    )
    bv_c = np.ascontiguousarray(
        np.concatenate([bv[h] for h in heads]).reshape(NPAIR, P).T, dtype=np.float32
    )
    wo_c = np.ascontiguousarray(
        Wo[:, g * HPC * DK : (g + 1) * HPC * DK].T, dtype=np.float16
    )
    return {
        "xT": xT,
        "wq": wq_c,
        "wk": wk_c,
        "wv": wv_c,
        "wo_t": wo_c,
        "bq": bq_c,
        "bk": bk_c,
        "bv": bv_c,
    }


def kernel(x, Wq, bq, Wk, bk, Wv, bv, Wo, bo, _trace=False, _tmpdir=None):
    x = np.asarray(x, dtype=np.float32)
    nc = _get_nc()
    in_maps = [
        _core_inputs(x, Wq, bq, Wk, bk, Wv, bv, Wo, c) for c in range(NCORES)
    ]
    kw = {}
    if _trace:
        kw = dict(trace=True, tmpdir=_tmpdir)
    res = bass_utils.run_bass_kernel_spmd(
        nc, in_maps, core_ids=list(range(NCORES)), **kw
    )
    bo = np.asarray(bo, dtype=np.float32)
    out = np.empty((B, S, D), dtype=np.float32)
    for b in range(B):
        part = res.results[2 * b]["out_part"].astype(np.float32) + res.results[
            2 * b + 1
        ]["out_part"].astype(np.float32)
        out[b] = part.T + bo
    if _trace:
        kernel._last_results = res
    return out


# revision 33
# speedup vs baseline: 1.0143x; 1.0143x over previous
"""Trainium2 Bass kernel for 16-head causal MultiHeadAttention.

Problem: x [4, 2048, 1024], 16 heads of dim 64, causal softmax attention,
output projection Wo [1024, 1024] + bo.

Sharding over 8 NeuronCores: core c handles batch b = c // 2 and head-group
g = c % 2 (8 heads each).  Each core computes its 8 heads' Q/K/V projections,
causal attention, and a partial output projection against its row-slice of
Wo.  The two cores of a batch return partial [D, S] outputs that the host
sums, transposes, and biases.

Schedule: the attention inner loop is software-pipelined per chunk (one
128-row t-tile, both heads of a pair side by side): scores matmuls ->
merged causally-trimmed exp (ACT) -> lag-1 AV matmuls, with a filler queue
of projection / V-transpose / output-projection work units drained between
chunks so the PE never idles while ACT runs exp.  Work units are j-ordered
so pair p's attention at s-block j only needs units (p, <=j), letting
pair-0 projections interleave with pair-0 attention (tiny prologue).

Score matmuls contract over dk=64, so the two heads' matmuls run
concurrently in 64x128 row-tiled PE mode (tile positions inferred from
the operands' base partitions); they are issued in batches of two chunks
(4 matmuls) per 64-mode stretch so the 128<->64 mode-switch drain is paid
once per batch.  A junk-transpose warm-up keeps the PE clock ramped
through the initial input-DMA wait; each j-block's softmax normalization
releases the single pa accumulator via one fast ACT copy, with
reciprocal+rescale running off the SBUF copy (reciprocal_approx_fast must
read partition offset 0).  Weights are pre-rearranged host-side so each
pair's projection weights load as plain 2KB-per-partition-line DMAs, and
the partial output is returned as f16 to halve the output-DMA tail.

PSUM (8 banks): sc [128,1024]x2 (scores, both heads) + pa [128,1024]x1
(AV accum + softmax denominators via ones-columns of V') + ps [128,512]x2
(projection groups / V-transposes / output projection / warm-up).
"""

import sys
from collections import deque

for _p in ("/opt/trn_rl_repo", "/root/.axon_site/_ro/trn_rl_repo"):
    if _p not in sys.path:
        sys.path.insert(0, _p)

import numpy as np

import concourse.bacc as bacc
import concourse.mybir as mybir
from concourse import bass_utils
from concourse.masks import make_identity, make_upper_triangular
from concourse.tile import TileContext

P = 128
S = 2048  # sequence length
D = 1024  # hidden size
H = 16  # total heads
DK = 64  # head dim
B = 4  # batch
NCORES = 8
HPC = 8  # heads per core
NPAIR = HPC // 2  # head pairs per core
SB = 512  # s-block width
NSB = S // SB  # 4
TT = S // P  # 16 t-tiles
DT = D // P  # 8 d-tiles

F32 = mybir.dt.float32
F16 = mybir.dt.float16
AF = mybir.ActivationFunctionType
MUL = mybir.AluOpType.mult


def build_nc(debug=False):
    nc = bacc.Bacc()
    xT = nc.dram_tensor("xT", [D, S], F16, kind="ExternalInput")
    # weights pre-rearranged host-side: [r, p*1024 + d*128 + c] =
    # W[d*128 + r, p*128 + c], so each pair loads as one plain
    # [128, 1024] DMA with 2KB partition lines
    wq = nc.dram_tensor("wq", [P, NPAIR * DT * P], F16, kind="ExternalInput")
    wk = nc.dram_tensor("wk", [P, NPAIR * DT * P], F16, kind="ExternalInput")
    wv = nc.dram_tensor("wv", [P, NPAIR * DT * P], F16, kind="ExternalInput")
    wo_t = nc.dram_tensor("wo_t", [HPC * DK, D], F16, kind="ExternalInput")
    bq = nc.dram_tensor("bq", [P, NPAIR], F32, kind="ExternalInput")
    bk = nc.dram_tensor("bk", [P, NPAIR], F32, kind="ExternalInput")
    bv = nc.dram_tensor("bv", [P, NPAIR], F32, kind="ExternalInput")
    out = nc.dram_tensor("out_part", [D, S], F16, kind="ExternalOutput")

    with TileContext(nc) as tc:
        from contextlib import ExitStack

        with ExitStack() as ctx:
            pool = lambda *a, **k: ctx.enter_context(tc.tile_pool(*a, **k))
            xt_pool = pool(name="xt", bufs=DT)
            wgt_pool = pool(name="wgt", bufs=6)
            wo_pool = pool(name="wo", bufs=NPAIR)
            qt_pool = pool(name="qt", bufs=2)
            kt_pool = pool(name="kt", bufs=2)
            vp_pool = pool(name="vp", bufs=2)
            vst_pool = pool(name="vst", bufs=2)
            wt_pool = pool(name="wt", bufs=4)
            ot_pool = pool(name="ot", bufs=NPAIR)
            den_pool = pool(name="den", bufs=2)
            rcs_pool = pool(name="rcs", bufs=2)
            ost_pool = pool(name="ost", bufs=3)
            const_pool = pool(name="const", bufs=1)
            ps_sc = pool(name="ps_sc", bufs=2, space="PSUM")
            ps_pa = pool(name="ps_pa", bufs=1, space="PSUM")
            ps_ps = pool(name="ps_ps", bufs=2, space="PSUM")

            # --- constants ---
            ident = const_pool.tile([P, P], F16)
            make_identity(nc, ident[:])
            # mask2[r, g*128+c] = 1 if c >= r else 0, for both head groups
            mask2 = const_pool.tile([P, 2 * P], F16)
            make_upper_triangular(nc, mask2[:, 0:P], val=1.0, diag=True)
            make_upper_triangular(nc, mask2[:, P : 2 * P], val=1.0, diag=True)
            bq_t = const_pool.tile([P, NPAIR], F32)
            nc.sync.dma_start(bq_t[:], bq[:])
            bk_t = const_pool.tile([P, NPAIR], F32)
            nc.sync.dma_start(bk_t[:], bk[:])
            bv_t = const_pool.tile([P, NPAIR], F32)
            nc.sync.dma_start(bv_t[:], bv[:])

            # --- resident inputs ---
            def load_wgt(srcw, p, name, splits=2):
                t = wgt_pool.tile([P, DT * P], F16, tag="wgt", name=name)
                h = DT * P // splits
                for u in range(splits):
                    nc.sync.dma_start(
                        t[:, u * h : (u + 1) * h],
                        srcw[:, p * DT * P + u * h : p * DT * P + (u + 1) * h],
                    )
                return t

            # PE warm-up: junk transposes during the initial input DMA so
            # the tensor engine is at full clock when real matmuls arrive
            for _ in range(25):
                wpt = ps_ps.tile([P, SB], F16, tag="ps", name="warm")
                for u in range(SB // P):
                    nc.tensor.transpose(
                        wpt[:, u * P : (u + 1) * P], ident[:], ident[:]
                    )

            wtiles = {}  # (p, nm) -> wgt tile
            wtiles[(0, "q")] = load_wgt(wq, 0, "wq0")
            wtiles[(0, "k")] = load_wgt(wk, 0, "wk0")
            wtiles[(0, "v")] = load_wgt(wv, 0, "wv0")
            # j-sliced xT load: the first half (s-blocks 0-1) lands first so
            # pair-0's early projection chains start well before the full
            # 4MB arrives
            xt = [
                xt_pool.tile([P, S], F16, tag="xt", name=f"xt{d}")
                for d in range(DT)
            ]
            for jj in range(0, NSB, 2):
                for d in range(DT):
                    nc.sync.dma_start(
                        xt[d][:, jj * SB : (jj + 2) * SB],
                        xT[d * P : (d + 1) * P, jj * SB : (jj + 2) * SB],
                    )

            # persistent V' tiles, one per in-flight pair; layout
            # [t, g*2048 + tg*128 + (0:64 V | 64:128 ones)]; the ones
            # halves are written once and survive slot reuse.
            vp_slots = []
            for s in range(2):
                v = vp_pool.tile([P, 2 * TT * P], F16, tag=f"vp{s}", bufs=1,
                                 name=f"vp{s}")
                nc.vector.memset(
                    v[:].rearrange("r (g tg c) -> r g tg c", g=2, tg=TT)[
                        :, :, :, DK:P
                    ],
                    1.0,
                )
                vp_slots.append(v)

            qt_tiles = {}
            kt_tiles = {}
            ot_tiles = []

            # ---- work units (filler between attention chunks) ----
            filler = deque()  # (key, closure)

            reserve = [0]  # units held back for the epilogue

            def drain(n):
                if len(filler) > 12:
                    n += 1
                while n > 0 and len(filler) > reserve[0]:
                    filler.popleft()[1]()
                    n -= 1

            def drain_until(key):
                while any(k <= key for k, _ in filler):
                    filler.popleft()[1]()

            def push_proj_unit(p, nm, j, dest_ap, bias_t):
                def unit():
                    ps = ps_ps.tile([P, SB], F32, tag="ps", name="ps_p")
                    w = wtiles[(p, nm)]
                    for d in range(DT):
                        nc.tensor.matmul(
                            ps[:],
                            w[:, d * P : (d + 1) * P],
                            xt[d][:, j * SB : (j + 1) * SB],
                            start=(d == 0),
                            stop=(d == DT - 1),
                        )
                    nc.vector.tensor_scalar_add(
                        dest_ap, ps[:], bias_t[:, p : p + 1],
                    )

                filler.append(((p, j), unit))

            def push_vtrans_unit(p, j, vst):
                def unit():
                    pt = ps_ps.tile([P, SB], F16, tag="ps", name="pt")
                    for u in range(SB // P):
                        nc.tensor.transpose(
                            pt[:, u * P : (u + 1) * P],
                            vst[:, u * P : (u + 1) * P],
                            ident[:],
                        )
                    # scatter [t, (u, g, c)] -> vp[t, (g, 4j+u, c)]
                    vpt = vp_slots[p % 2]
                    dst = vpt[:].rearrange(
                        "r (g tg c) -> r g tg c", g=2, tg=TT
                    )[:, :, 4 * j : 4 * j + 4, 0:DK]
                    src = pt[:].rearrange(
                        "r (u g c) -> r g u c", u=SB // P, g=2
                    )
                    nc.vector.tensor_copy(dst, src)

                filler.append(((p, j), unit))

            def push_pair_units(p):
                qt = qt_pool.tile([P, S], F16, tag="qt", name=f"qt{p}")
                kt = kt_pool.tile([P, S], F16, tag="kt", name=f"kt{p}")
                qt_tiles[p] = qt
                kt_tiles[p] = kt
                if p > 0:
                    for nm, srcw in (("q", wq), ("k", wk), ("v", wv)):
                        wtiles[(p, nm)] = load_wgt(srcw, p, f"w{nm}{p}")
                # vt-j is pushed one unit after its v-j so the transposes
                # don't immediately wait on the DVE bias-copy of vst
                pend_vt = None
                for j in range(NSB):
                    push_proj_unit(p, "q", j, qt[:, j * SB : (j + 1) * SB],
                                   bq_t)
                    if pend_vt is not None:
                        push_vtrans_unit(p, pend_vt[0], pend_vt[1])
                    push_proj_unit(p, "k", j, kt[:, j * SB : (j + 1) * SB],
                                   bk_t)
                    vst = vst_pool.tile([P, SB], F16, tag="vst", name="vst")
                    push_proj_unit(p, "v", j, vst[:], bv_t)
                    pend_vt = (j, vst)
                push_vtrans_unit(p, pend_vt[0], pend_vt[1])

            def push_oproj_unit(m, j, wo_tiles):
                def unit():
                    ps = ps_ps.tile([P, SB], F32, tag="ps", name="ps_o")
                    for p in range(NPAIR):
                        nc.tensor.matmul(
                            ps[:],
                            wo_tiles[p][:, m * P : (m + 1) * P],
                            ot_tiles[p][:, j * SB : (j + 1) * SB],
                            start=(p == 0),
                            stop=(p == NPAIR - 1),
                        )
                    st = ost_pool.tile([P, SB], F16, tag="ost", name="ost")
                    nc.vector.tensor_copy(st[:], ps[:])
                    nc.sync.dma_start(
                        out[m * P : (m + 1) * P, j * SB : (j + 1) * SB],
                        st[:],
                    )

                filler.append(((NPAIR, j, m), unit))

            # ---- main schedule ----
            push_pair_units(0)
            wo_tiles = []
            for p in range(NPAIR):
                qt = qt_tiles[p]
                kt = kt_tiles[p]
                vpt = vp_slots[p % 2]
                ot = ot_pool.tile([P, S], F16, tag="ot", name=f"ot{p}")
                ot_tiles.append(ot)
                if p + 1 < NPAIR:
                    push_pair_units(p + 1)
                else:
                    # wo loads land during pair-3 attention; hold back a few
                    # filler units so the epilogue has ready PE work to
                    # overlap the final rescale chain
                    reserve[0] = 6
                    for pp in range(NPAIR):
                        t = wo_pool.tile([P, D], F16, tag="wo", name=f"wo{pp}")
                        nc.sync.dma_start(t[:], wo_t[pp * P : (pp + 1) * P, :])
                        wo_tiles.append(t)

                for j in range(NSB):
                    drain_until((p, j))
                    nt = 4 * j + 4
                    pa = ps_pa.tile([P, 2 * SB], F32, tag="pa", name="pa")
                    pend = []

                    def issue_av(item):
                        i, wt, c0 = item
                        for g in range(2):
                            nc.tensor.matmul(
                                pa[:, g * SB + c0 : (g + 1) * SB],
                                vpt[:, g * TT * P + i * P : g * TT * P + (i + 1) * P],
                                wt[:, g * SB + c0 : (g + 1) * SB],
                                start=(i == 0),
                                stop=(i == nt - 1),
                            )

                    # two chunks per batch: the four 64-contraction score
                    # matmuls run in one 64x128 row-tiled PE stretch (heads
                    # at tile rows 0/64 execute concurrently), paying the
                    # 128<->64 mode-switch drain once per batch.
                    for i2 in range(0, nt, 2):
                        batch = []
                        for i in (i2, i2 + 1):
                            r = i - 4 * j
                            c0 = P * max(r, 0)
                            sc = ps_sc.tile([P, 2 * SB], F32, tag="sc",
                                            name="sc")
                            for g in range(2):
                                nc.tensor.matmul(
                                    sc[:, g * SB + c0 : (g + 1) * SB],
                                    kt[g * DK : (g + 1) * DK,
                                       i * P : (i + 1) * P],
                                    qt[g * DK : (g + 1) * DK,
                                       j * SB + c0 : (j + 1) * SB],
                                    start=True,
                                    stop=True,
                                )
                            batch.append((i, sc, c0))
                        new_pend = []
                        for i, sc, c0 in batch:
                            r = i - 4 * j
                            wt = wt_pool.tile([P, 2 * SB], F16, tag="wt",
                                              name="wt")
                            if r >= 0:
                                scv = sc[:].rearrange(
                                    "r (g w) -> r g w", g=2
                                )[:, :, c0:]
                                wtv = wt[:].rearrange(
                                    "r (g w) -> r g w", g=2
                                )[:, :, c0:]
                            else:
                                scv = sc[:]
                                wtv = wt[:]
                            nc.scalar.activation(wtv, scv, AF.Exp,
                                                 scale=0.125)
                            if r >= 0:
                                wmv = wt[:].rearrange(
                                    "r (g w) -> r g w", g=2
                                )[:, :, c0 : c0 + P]
                                nc.vector.tensor_tensor(
                                    wmv,
                                    wmv,
                                    mask2[:].rearrange(
                                        "r (g w) -> r g w", g=2
                                    ),
                                    MUL,
                                )
                            new_pend.append((i, wt, c0))
                        drain(1)
                        for item in pend:
                            issue_av(item)
                        pend = new_pend
                        drain(1)
                    for item in pend:
                        issue_av(item)
                    # normalize: ones-columns of V' put the softmax
                    # denominator in pa rows 64:127 (replicated).  One fast
                    # ACT copy releases the single pa buffer; recip + muls
                    # then run off the SBUF copy without blocking the PE.
                    pac = den_pool.tile([P, 2 * SB], F32, tag="den",
                                        name="pac")
                    nc.scalar.copy(pac[:], pa[:])
                    den = den_pool.tile([DK, 2 * SB], F32, tag="dend",
                                        name="den")
                    nc.vector.tensor_copy(den[:], pac[DK:P, :])
                    rcs = rcs_pool.tile([DK, 2 * SB], F32, tag="rcs",
                                        name="rcs")
                    nc.vector.reciprocal_approx_fast(rcs[:], den[:])
                    for g in range(2):
                        nc.vector.tensor_tensor(
                            ot[g * DK : (g + 1) * DK, j * SB : (j + 1) * SB],
                            pac[0:DK, g * SB : (g + 1) * SB],
                            rcs[:, g * SB : (g + 1) * SB],
                            MUL,
                        )
                    if p == NPAIR - 1:
                        for m in range(DT):
                            push_oproj_unit(m, j, wo_tiles)
                    drain(1)

            # epilogue: drain remaining output-projection units
            while filler:
                filler.popleft()[1]()

    nc.compile()
    return nc


_NC_CACHE = None


def _get_nc():
    global _NC_CACHE
    if _NC_CACHE is None:
        _NC_CACHE = build_nc()
    return _NC_CACHE


def _core_inputs(x, Wq, bq, Wk, bk, Wv, bv, Wo, c):
    b, g = c // 2, c % 2
    heads = range(g * HPC, (g + 1) * HPC)
    xT = np.ascontiguousarray(x[b].T, dtype=np.float16)
    def warr(W):
        w = np.concatenate([W[h] for h in heads], axis=1)  # [D, 512]
        # [r, p*1024 + d*128 + c] = w[d*128 + r, p*128 + c]
        blocks = [
            w[:, p * P : (p + 1) * P]
            .reshape(DT, P, P)
            .transpose(1, 0, 2)
            .reshape(P, DT * P)
            for p in range(NPAIR)
        ]
        return np.ascontiguousarray(
            np.concatenate(blocks, axis=1), dtype=np.float16
        )

    wq_c = warr(Wq)
    wk_c = warr(Wk)
    wv_c = warr(Wv)
    bq_c = np.ascontiguousarray(
        np.concatenate([bq[h] for h in heads]).reshape(NPAIR, P).T, dtype=np.float32
    )
    bk_c = np.ascontiguousarray(
        np.concatenate([bk[h] for h in heads]).reshape(NPAIR, P).T, dtype=np.float32
    )
    bv_c = np.ascontiguousarray(
        np.concatenate([bv[h] for h in heads]).reshape(NPAIR, P).T, dtype=np.float32
    )
    wo_c = np.ascontiguousarray(
        Wo[:, g * HPC * DK : (g + 1) * HPC * DK].T, dtype=np.float16
    )
    return {
        "xT": xT,
        "wq": wq_c,
        "wk": wk_c,
        "wv": wv_c,
        "wo_t": wo_c,
        "bq": bq_c,
        "bk": bk_c,
        "bv": bv_c,
    }


def kernel(x, Wq, bq, Wk, bk, Wv, bv, Wo, bo, _trace=False, _tmpdir=None):
    x = np.asarray(x, dtype=np.float32)
    nc = _get_nc()
    in_maps = [
        _core_inputs(x, Wq, bq, Wk, bk, Wv, bv, Wo, c) for c in range(NCORES)
    ]
    kw = {}
    if _trace:
        kw = dict(trace=True, tmpdir=_tmpdir)
    res = bass_utils.run_bass_kernel_spmd(
        nc, in_maps, core_ids=list(range(NCORES)), **kw
    )
    bo = np.asarray(bo, dtype=np.float32)
    out = np.empty((B, S, D), dtype=np.float32)
    for b in range(B):
        part = res.results[2 * b]["out_part"].astype(np.float32) + res.results[
            2 * b + 1
        ]["out_part"].astype(np.float32)
        out[b] = part.T + bo
    if _trace:
        kernel._last_results = res
    return out


# revision 35
# speedup vs baseline: 1.0436x; 1.0290x over previous
"""Trainium2 Bass kernel for 16-head causal MultiHeadAttention.

Problem: x [4, 2048, 1024], 16 heads of dim 64, causal softmax attention,
output projection Wo [1024, 1024] + bo.

Sharding over 8 NeuronCores: core c handles batch b = c // 2 and head-group
g = c % 2 (8 heads each).  Each core computes its 8 heads' Q/K/V projections,
causal attention, and a partial output projection against its row-slice of
Wo.  The two cores of a batch return partial [D, S] outputs that the host
sums, transposes, and biases.

Schedule: the attention inner loop is software-pipelined per chunk (one
128-row t-tile, both heads of a pair side by side): scores matmuls ->
merged causally-trimmed exp (ACT) -> lag-1 AV matmuls, with a filler queue
of projection / V-transpose / output-projection work units drained between
chunks so the PE never idles while ACT runs exp.  Work units are j-ordered
so pair p's attention at s-block j only needs units (p, <=j), letting
pair-0 projections interleave with pair-0 attention (tiny prologue).

Score matmuls contract over dk=64, so the two heads' matmuls run
concurrently in 64x128 row-tiled PE mode (tile positions inferred from
the operands' base partitions); they are issued in batches of two chunks
(4 matmuls) per 64-mode stretch so the 128<->64 mode-switch drain is paid
once per batch.  A junk-transpose warm-up keeps the PE clock ramped
through the initial input-DMA wait; each j-block's softmax normalization
releases the single pa accumulator via one fast ACT copy, with
reciprocal+rescale running off the SBUF copy (reciprocal_approx_fast must
read partition offset 0).  Weights are pre-rearranged host-side so each
pair's projection weights load as plain 2KB-per-partition-line DMAs, and
the partial output is returned as f16 to halve the output-DMA tail.

PSUM (8 banks): sc [128,1024]x2 (scores, both heads) + pa [128,1024]x1
(AV accum + softmax denominators via ones-columns of V') + ps [128,512]x2
(projection groups / V-transposes / output projection / warm-up).
"""

import sys
from collections import deque

for _p in ("/opt/trn_rl_repo", "/root/.axon_site/_ro/trn_rl_repo"):
    if _p not in sys.path:
        sys.path.insert(0, _p)

import numpy as np

import concourse.bacc as bacc
import concourse.mybir as mybir
from concourse import bass_utils
from concourse.masks import make_identity, make_upper_triangular
from concourse.tile import TileContext

P = 128
S = 2048  # sequence length
D = 1024  # hidden size
H = 16  # total heads
DK = 64  # head dim
B = 4  # batch
NCORES = 8
HPC = 8  # heads per core
NPAIR = HPC // 2  # head pairs per core
SB = 512  # s-block width
NSB = S // SB  # 4
TT = S // P  # 16 t-tiles
DT = D // P  # 8 d-tiles

F32 = mybir.dt.float32
F16 = mybir.dt.float16
AF = mybir.ActivationFunctionType
MUL = mybir.AluOpType.mult


def build_nc(debug=False):
    nc = bacc.Bacc()
    xT = nc.dram_tensor("xT", [D, S], F16, kind="ExternalInput")
    # weights pre-rearranged host-side: [r, p*1024 + d*128 + c] =
    # W[d*128 + r, p*128 + c], so each pair loads as one plain
    # [128, 1024] DMA with 2KB partition lines
    wq = nc.dram_tensor("wq", [P, NPAIR * DT * P], F16, kind="ExternalInput")
    wk = nc.dram_tensor("wk", [P, NPAIR * DT * P], F16, kind="ExternalInput")
    wv = nc.dram_tensor("wv", [P, NPAIR * DT * P], F16, kind="ExternalInput")
    wo_t = nc.dram_tensor("wo_t", [HPC * DK, D], F16, kind="ExternalInput")
    bq = nc.dram_tensor("bq", [P, NPAIR], F32, kind="ExternalInput")
    bk = nc.dram_tensor("bk", [P, NPAIR], F32, kind="ExternalInput")
    bv = nc.dram_tensor("bv", [P, NPAIR], F32, kind="ExternalInput")
    out = nc.dram_tensor("out_part", [D, S], F16, kind="ExternalOutput")

    with TileContext(nc) as tc:
        from contextlib import ExitStack

        with ExitStack() as ctx:
            pool = lambda *a, **k: ctx.enter_context(tc.tile_pool(*a, **k))
            xt_pool = pool(name="xt", bufs=DT)
            wgt_pool = pool(name="wgt", bufs=6)
            wo_pool = pool(name="wo", bufs=NPAIR)
            qt_pool = pool(name="qt", bufs=2)
            kt_pool = pool(name="kt", bufs=2)
            vp_pool = pool(name="vp", bufs=2)
            vst_pool = pool(name="vst", bufs=2)
            wt_pool = pool(name="wt", bufs=4)
            ot_pool = pool(name="ot", bufs=NPAIR)
            den_pool = pool(name="den", bufs=2)
            rcs_pool = pool(name="rcs", bufs=2)
            ost_pool = pool(name="ost", bufs=3)
            const_pool = pool(name="const", bufs=1)
            ps_sc = pool(name="ps_sc", bufs=2, space="PSUM")
            ps_pa = pool(name="ps_pa", bufs=1, space="PSUM")
            ps_ps = pool(name="ps_ps", bufs=2, space="PSUM")

            # --- constants ---
            ident = const_pool.tile([P, P], F16)
            make_identity(nc, ident[:])
            # mask2[r, g*128+c] = 1 if c >= r else 0, for both head groups
            mask2 = const_pool.tile([P, 2 * P], F16)
            make_upper_triangular(nc, mask2[:, 0:P], val=1.0, diag=True)
            make_upper_triangular(nc, mask2[:, P : 2 * P], val=1.0, diag=True)
            bq_t = const_pool.tile([P, NPAIR], F32)
            nc.sync.dma_start(bq_t[:], bq[:])
            bk_t = const_pool.tile([P, NPAIR], F32)
            nc.sync.dma_start(bk_t[:], bk[:])
            bv_t = const_pool.tile([P, NPAIR], F32)
            nc.sync.dma_start(bv_t[:], bv[:])

            # --- resident inputs ---
            def load_wgt(srcw, p, name, splits=2):
                t = wgt_pool.tile([P, DT * P], F16, tag="wgt", name=name)
                h = DT * P // splits
                for u in range(splits):
                    nc.sync.dma_start(
                        t[:, u * h : (u + 1) * h],
                        srcw[:, p * DT * P + u * h : p * DT * P + (u + 1) * h],
                    )
                return t

            # PE warm-up: junk transposes during the initial input DMA so
            # the tensor engine is at full clock when real matmuls arrive
            for _ in range(25):
                wpt = ps_ps.tile([P, SB], F16, tag="ps", name="warm")
                for u in range(SB // P):
                    nc.tensor.transpose(
                        wpt[:, u * P : (u + 1) * P], ident[:], ident[:]
                    )

            wtiles = {}  # (p, nm) -> wgt tile
            wtiles[(0, "q")] = load_wgt(wq, 0, "wq0")
            wtiles[(0, "k")] = load_wgt(wk, 0, "wk0")
            wtiles[(0, "v")] = load_wgt(wv, 0, "wv0")
            # j-sliced xT load: the first half (s-blocks 0-1) lands first so
            # pair-0's early projection chains start well before the full
            # 4MB arrives
            xt = [
                xt_pool.tile([P, S], F16, tag="xt", name=f"xt{d}")
                for d in range(DT)
            ]
            for jj in range(0, NSB, 2):
                for d in range(DT):
                    nc.sync.dma_start(
                        xt[d][:, jj * SB : (jj + 2) * SB],
                        xT[d * P : (d + 1) * P, jj * SB : (jj + 2) * SB],
                    )

            # persistent V' tiles, one per in-flight pair; layout
            # [t, g*2048 + tg*128 + (0:64 V | 64:128 ones)]; the ones
            # halves are written once and survive slot reuse.
            vp_slots = []
            for s in range(2):
                v = vp_pool.tile([P, 2 * TT * P], F16, tag=f"vp{s}", bufs=1,
                                 name=f"vp{s}")
                nc.vector.memset(
                    v[:].rearrange("r (g tg c) -> r g tg c", g=2, tg=TT)[
                        :, :, :, DK:P
                    ],
                    1.0,
                )
                vp_slots.append(v)

            qt_tiles = {}
            kt_tiles = {}
            ot_tiles = []

            # ---- work units (filler between attention chunks) ----
            filler = deque()  # (key, closure)

            reserve = [0]  # units held back for the epilogue

            def drain(n):
                if len(filler) > 12:
                    n += 1
                while n > 0 and len(filler) > reserve[0]:
                    filler.popleft()[1]()
                    n -= 1

            def drain_until(key):
                while any(k <= key for k, _ in filler):
                    filler.popleft()[1]()

            def push_proj_unit(p, nm, j, dest_ap, bias_t):
                def unit():
                    ps = ps_ps.tile([P, SB], F32, tag="ps", name="ps_p")
                    w = wtiles[(p, nm)]
                    for d in range(DT):
                        nc.tensor.matmul(
                            ps[:],
                            w[:, d * P : (d + 1) * P],
                            xt[d][:, j * SB : (j + 1) * SB],
                            start=(d == 0),
                            stop=(d == DT - 1),
                        )
                    nc.vector.tensor_scalar_add(
                        dest_ap, ps[:], bias_t[:, p : p + 1],
                    )

                filler.append(((p, j), unit))

            def push_vtrans_unit(p, j, vst):
                def unit():
                    pt = ps_ps.tile([P, SB], F16, tag="ps", name="pt")
                    for u in range(SB // P):
                        nc.tensor.transpose(
                            pt[:, u * P : (u + 1) * P],
                            vst[:, u * P : (u + 1) * P],
                            ident[:],
                        )
                    # scatter [t, (u, g, c)] -> vp[t, (g, 4j+u, c)]
                    vpt = vp_slots[p % 2]
                    dst = vpt[:].rearrange(
                        "r (g tg c) -> r g tg c", g=2, tg=TT
                    )[:, :, 4 * j : 4 * j + 4, 0:DK]
                    src = pt[:].rearrange(
                        "r (u g c) -> r g u c", u=SB // P, g=2
                    )
                    nc.vector.tensor_copy(dst, src)

                filler.append(((p, j), unit))

            def push_pair_units(p):
                qt = qt_pool.tile([P, S], F16, tag="qt", name=f"qt{p}")
                kt = kt_pool.tile([P, S], F16, tag="kt", name=f"kt{p}")
                qt_tiles[p] = qt
                kt_tiles[p] = kt
                if p > 0:
                    for nm, srcw in (("q", wq), ("k", wk), ("v", wv)):
                        wtiles[(p, nm)] = load_wgt(srcw, p, f"w{nm}{p}")
                # vt-j is pushed one unit after its v-j so the transposes
                # don't immediately wait on the DVE bias-copy of vst
                pend_vt = None
                for j in range(NSB):
                    push_proj_unit(p, "q", j, qt[:, j * SB : (j + 1) * SB],
                                   bq_t)
                    if pend_vt is not None:
                        push_vtrans_unit(p, pend_vt[0], pend_vt[1])
                    push_proj_unit(p, "k", j, kt[:, j * SB : (j + 1) * SB],
                                   bk_t)
                    vst = vst_pool.tile([P, SB], F16, tag="vst", name="vst")
                    push_proj_unit(p, "v", j, vst[:], bv_t)
                    pend_vt = (j, vst)
                push_vtrans_unit(p, pend_vt[0], pend_vt[1])

            def push_oproj_unit(m, j, wo_tiles):
                def unit():
                    ps = ps_ps.tile([P, SB], F32, tag="ps", name="ps_o")
                    for p in range(NPAIR):
                        nc.tensor.matmul(
                            ps[:],
                            wo_tiles[p][:, m * P : (m + 1) * P],
                            ot_tiles[p][:, j * SB : (j + 1) * SB],
                            start=(p == 0),
                            stop=(p == NPAIR - 1),
                        )
                    st = ost_pool.tile([P, SB], F16, tag="ost", name="ost")
                    nc.vector.tensor_copy(st[:], ps[:])
                    nc.sync.dma_start(
                        out[m * P : (m + 1) * P, j * SB : (j + 1) * SB],
                        st[:],
                    )

                filler.append(((NPAIR, j, m), unit))

            # ---- main schedule ----
            push_pair_units(0)
            wo_tiles = []
            for p in range(NPAIR):
                qt = qt_tiles[p]
                kt = kt_tiles[p]
                vpt = vp_slots[p % 2]
                ot = ot_pool.tile([P, S], F16, tag="ot", name=f"ot{p}")
                ot_tiles.append(ot)
                if p + 1 < NPAIR:
                    push_pair_units(p + 1)
                else:
                    # wo loads land during pair-3 attention; hold back a few
                    # filler units so the epilogue has ready PE work to
                    # overlap the final rescale chain
                    reserve[0] = 6
                    for pp in range(NPAIR):
                        t = wo_pool.tile([P, D], F16, tag="wo", name=f"wo{pp}")
                        nc.sync.dma_start(t[:], wo_t[pp * P : (pp + 1) * P, :])
                        wo_tiles.append(t)

                # close(j): final AV flush + softmax normalization for the
                # previous j.  Deferred until after the next j's first score
                # batch so the boundary is filled with PE work while the
                # pa-release copy runs.
                close_state = [None]  # (j, pa, pend)

                def issue_av(pa, nt, item):
                    i, wt, c0 = item
                    for g in range(2):
                        nc.tensor.matmul(
                            pa[:, g * SB + c0 : (g + 1) * SB],
                            vpt[:, g * TT * P + i * P : g * TT * P + (i + 1) * P],
                            wt[:, g * SB + c0 : (g + 1) * SB],
                            start=(i == 0),
                            stop=(i == nt - 1),
                        )

                def close_j():
                    if close_state[0] is None:
                        return
                    j, pa, pend_items = close_state[0]
                    close_state[0] = None
                    nt = 4 * j + 4
                    for item in pend_items:
                        issue_av(pa, nt, item)
                    pac = den_pool.tile([P, 2 * SB], F32, tag="den",
                                        name="pac")
                    nc.scalar.copy(pac[:], pa[:])
                    den = den_pool.tile([DK, 2 * SB], F32, tag="dend",
                                        name="den")
                    nc.vector.tensor_copy(den[:], pac[DK:P, :])
                    rcs = rcs_pool.tile([DK, 2 * SB], F32, tag="rcs",
                                        name="rcs")
                    nc.vector.reciprocal_approx_fast(rcs[:], den[:])
                    for g in range(2):
                        nc.vector.tensor_tensor(
                            ot[g * DK : (g + 1) * DK, j * SB : (j + 1) * SB],
                            pac[0:DK, g * SB : (g + 1) * SB],
                            rcs[:, g * SB : (g + 1) * SB],
                            MUL,
                        )
                    if p == NPAIR - 1:
                        for m in range(DT):
                            push_oproj_unit(m, j, wo_tiles)
                    drain(1)

                for j in range(NSB):
                    drain_until((p, j))
                    nt = 4 * j + 4
                    pa = None
                    pend = []

                    # two chunks per batch: the four 64-contraction score
                    # matmuls run in one 64x128 row-tiled PE stretch (heads
                    # at tile rows 0/64 execute concurrently), paying the
                    # 128<->64 mode-switch drain once per batch.
                    for i2 in range(0, nt, 2):
                        batch = []
                        for i in (i2, i2 + 1):
                            r = i - 4 * j
                            c0 = P * max(r, 0)
                            sc = ps_sc.tile([P, 2 * SB], F32, tag="sc",
                                            name="sc")
                            for g in range(2):
                                nc.tensor.matmul(
                                    sc[:, g * SB + c0 : (g + 1) * SB],
                                    kt[g * DK : (g + 1) * DK,
                                       i * P : (i + 1) * P],
                                    qt[g * DK : (g + 1) * DK,
                                       j * SB + c0 : (j + 1) * SB],
                                    start=True,
                                    stop=True,
                                )
                            batch.append((i, sc, c0))
                        new_pend = []
                        for i, sc, c0 in batch:
                            r = i - 4 * j
                            wt = wt_pool.tile([P, 2 * SB], F16, tag="wt",
                                              name="wt")
                            if r >= 0:
                                scv = sc[:].rearrange(
                                    "r (g w) -> r g w", g=2
                                )[:, :, c0:]
                                wtv = wt[:].rearrange(
                                    "r (g w) -> r g w", g=2
                                )[:, :, c0:]
                            else:
                                scv = sc[:]
                                wtv = wt[:]
                            nc.scalar.activation(wtv, scv, AF.Exp,
                                                 scale=0.125)
                            if r >= 0:
                                wmv = wt[:].rearrange(
                                    "r (g w) -> r g w", g=2
                                )[:, :, c0 : c0 + P]
                                nc.vector.tensor_tensor(
                                    wmv,
                                    wmv,
                                    mask2[:].rearrange(
                                        "r (g w) -> r g w", g=2
                                    ),
                                    MUL,
                                )
                            new_pend.append((i, wt, c0))
                        if i2 == 0:
                            # previous j's final AVs + rescale run behind
                            # this batch's scores/exps
                            close_j()
                            pa = ps_pa.tile([P, 2 * SB], F32, tag="pa",
                                            name="pa")
                        else:
                            drain(1)
                            for item in pend:
                                issue_av(pa, nt, item)
                        pend = new_pend
                        drain(1)
                    close_state[0] = (j, pa, pend)
                # close the pair's last j before moving on
                close_j()

            # epilogue: drain remaining output-projection units
            while filler:
                filler.popleft()[1]()

    nc.compile()
    return nc


_NC_CACHE = None


def _get_nc():
    global _NC_CACHE
    if _NC_CACHE is None:
        _NC_CACHE = build_nc()
    return _NC_CACHE


def _core_inputs(x, Wq, bq, Wk, bk, Wv, bv, Wo, c):
    b, g = c // 2, c % 2
    heads = range(g * HPC, (g + 1) * HPC)
    xT = np.ascontiguousarray(x[b].T, dtype=np.float16)
    def warr(W):
        w = np.concatenate([W[h] for h in heads], axis=1)  # [D, 512]
        # [r, p*1024 + d*128 + c] = w[d*128 + r, p*128 + c]
        blocks = [
            w[:, p * P : (p + 1) * P]
            .reshape(DT, P, P)
            .transpose(1, 0, 2)
            .reshape(P, DT * P)
            for p in range(NPAIR)
        ]
        return np.ascontiguousarray(
            np.concatenate(blocks, axis=1), dtype=np.float16
        )

    wq_c = warr(Wq)
    wk_c = warr(Wk)
    wv_c = warr(Wv)
    bq_c = np.ascontiguousarray(
        np.concatenate([bq[h] for h in heads]).reshape(NPAIR, P).T, dtype=np.float32
    )
    bk_c = np.ascontiguousarray(
        np.concatenate([bk[h] for h in heads]).reshape(NPAIR, P).T, dtype=np.float32
    )
    bv_c = np.ascontiguousarray(
        np.concatenate([bv[h] for h in heads]).reshape(NPAIR, P).T, dtype=np.float32
    )
    wo_c = np.ascontiguousarray(
        Wo[:, g * HPC * DK : (g + 1) * HPC * DK].T, dtype=np.float16
    )
    return {
        "xT": xT,
        "wq": wq_c,
        "wk": wk_c,
        "wv": wv_c,
        "wo_t": wo_c,
        "bq": bq_c,
        "bk": bk_c,
        "bv": bv_c,
    }


def kernel(x, Wq, bq, Wk, bk, Wv, bv, Wo, bo, _trace=False, _tmpdir=None):
    x = np.asarray(x, dtype=np.float32)
    nc = _get_nc()
    in_maps = [
        _core_inputs(x, Wq, bq, Wk, bk, Wv, bv, Wo, c) for c in range(NCORES)
    ]
    kw = {}
    if _trace:
        kw = dict(trace=True, tmpdir=_tmpdir)
    res = bass_utils.run_bass_kernel_spmd(
        nc, in_maps, core_ids=list(range(NCORES)), **kw
    )
    bo = np.asarray(bo, dtype=np.float32)
    out = np.empty((B, S, D), dtype=np.float32)
    for b in range(B):
        part = res.results[2 * b]["out_part"].astype(np.float32) + res.results[
            2 * b + 1
        ]["out_part"].astype(np.float32)
        out[b] = part.T + bo
    if _trace:
        kernel._last_results = res
    return out


# revision 39
# speedup vs baseline: 1.0571x; 1.0129x over previous
"""Trainium2 Bass kernel for 16-head causal MultiHeadAttention.

Problem: x [4, 2048, 1024], 16 heads of dim 64, causal softmax attention,
output projection Wo [1024, 1024] + bo.

Sharding over 8 NeuronCores: core c handles batch b = c // 2 and head-group
g = c % 2 (8 heads each).  Each core computes its 8 heads' Q/K/V projections,
causal attention, and a partial output projection against its row-slice of
Wo.  The two cores of a batch return partial [D, S] outputs that the host
sums, transposes, and biases.

Schedule: the attention inner loop is software-pipelined per chunk (one
128-row t-tile, both heads of a pair side by side): scores matmuls ->
merged causally-trimmed exp (ACT) -> lag-1 AV matmuls, with a filler queue
of projection / V-transpose / output-projection work units drained between
chunks so the PE never idles while ACT runs exp.  Work units are j-ordered
so pair p's attention at s-block j only needs units (p, <=j), letting
pair-0 projections interleave with pair-0 attention (tiny prologue).

Score matmuls contract over dk=64, so the two heads' matmuls run
concurrently in 64x128 row-tiled PE mode (tile positions inferred from
the operands' base partitions); they are issued in batches of two chunks
(4 matmuls) per 64-mode stretch so the 128<->64 mode-switch drain is paid
once per batch.  A junk-transpose warm-up keeps the PE clock ramped
through the initial input-DMA wait; each j-block's softmax normalization
releases the single pa accumulator via one fast ACT copy, with
reciprocal+rescale running off the SBUF copy (reciprocal_approx_fast must
read partition offset 0).  Weights are pre-rearranged host-side so each
pair's projection weights load as plain 2KB-per-partition-line DMAs, and
the partial output is returned as f16 to halve the output-DMA tail.

PSUM (8 banks): sc [128,1024]x2 (scores, both heads) + pa [128,1024]x1
(AV accum + softmax denominators via ones-columns of V') + ps [128,512]x2
(projection groups / V-transposes / output projection / warm-up).
"""

import sys
from collections import deque

for _p in ("/opt/trn_rl_repo", "/root/.axon_site/_ro/trn_rl_repo"):
    if _p not in sys.path:
        sys.path.insert(0, _p)

import numpy as np

import concourse.bacc as bacc
import concourse.mybir as mybir
from concourse import bass_utils
from concourse.masks import make_identity, make_upper_triangular
from concourse.tile import TileContext

P = 128
S = 2048  # sequence length
D = 1024  # hidden size
H = 16  # total heads
DK = 64  # head dim
B = 4  # batch
NCORES = 8
HPC = 8  # heads per core
NPAIR = HPC // 2  # head pairs per core
SB = 512  # s-block width
NSB = S // SB  # 4
TT = S // P  # 16 t-tiles
DT = D // P  # 8 d-tiles

F32 = mybir.dt.float32
F16 = mybir.dt.float16
AF = mybir.ActivationFunctionType
MUL = mybir.AluOpType.mult


def build_nc(debug=False):
    nc = bacc.Bacc()
    xT = nc.dram_tensor("xT", [D, S], F16, kind="ExternalInput")
    # weights pre-rearranged host-side: [r, p*1024 + d*128 + c] =
    # W[d*128 + r, p*128 + c], so each pair loads as one plain
    # [128, 1024] DMA with 2KB partition lines
    wq = nc.dram_tensor("wq", [P, NPAIR * DT * P], F16, kind="ExternalInput")
    wk = nc.dram_tensor("wk", [P, NPAIR * DT * P], F16, kind="ExternalInput")
    wv = nc.dram_tensor("wv", [P, NPAIR * DT * P], F16, kind="ExternalInput")
    wo_t = nc.dram_tensor("wo_t", [HPC * DK, D], F16, kind="ExternalInput")
    bq = nc.dram_tensor("bq", [P, NPAIR], F32, kind="ExternalInput")
    bk = nc.dram_tensor("bk", [P, NPAIR], F32, kind="ExternalInput")
    bv = nc.dram_tensor("bv", [P, NPAIR], F32, kind="ExternalInput")
    out = nc.dram_tensor("out_part", [D, S], F16, kind="ExternalOutput")

    with TileContext(nc) as tc:
        from contextlib import ExitStack

        with ExitStack() as ctx:
            pool = lambda *a, **k: ctx.enter_context(tc.tile_pool(*a, **k))
            xt_pool = pool(name="xt", bufs=DT)
            wgt_pool = pool(name="wgt", bufs=6)
            wo_pool = pool(name="wo", bufs=NPAIR)
            qt_pool = pool(name="qt", bufs=2)
            kt_pool = pool(name="kt", bufs=2)
            vp_pool = pool(name="vp", bufs=2)
            vst_pool = pool(name="vst", bufs=2)
            wt_pool = pool(name="wt", bufs=6)
            ot_pool = pool(name="ot", bufs=NPAIR)
            den_pool = pool(name="den", bufs=2)
            rcs_pool = pool(name="rcs", bufs=2)
            ost_pool = pool(name="ost", bufs=3)
            const_pool = pool(name="const", bufs=1)
            ps_sc = pool(name="ps_sc", bufs=2, space="PSUM")
            ps_pa = pool(name="ps_pa", bufs=1, space="PSUM")
            ps_ps = pool(name="ps_ps", bufs=2, space="PSUM")

            # --- constants ---
            ident = const_pool.tile([P, P], F16)
            make_identity(nc, ident[:])
            # mask2[r, g*128+c] = 1 if c >= r else 0, for both head groups
            mask2 = const_pool.tile([P, 2 * P], F16)
            make_upper_triangular(nc, mask2[:, 0:P], val=1.0, diag=True)
            make_upper_triangular(nc, mask2[:, P : 2 * P], val=1.0, diag=True)
            bq_t = const_pool.tile([P, NPAIR], F32)
            nc.sync.dma_start(bq_t[:], bq[:])
            bk_t = const_pool.tile([P, NPAIR], F32)
            nc.sync.dma_start(bk_t[:], bk[:])
            bv_t = const_pool.tile([P, NPAIR], F32)
            nc.sync.dma_start(bv_t[:], bv[:])

            # --- resident inputs ---
            def load_wgt(srcw, p, name, splits=2):
                t = wgt_pool.tile([P, DT * P], F16, tag="wgt", name=name)
                h = DT * P // splits
                for u in range(splits):
                    nc.sync.dma_start(
                        t[:, u * h : (u + 1) * h],
                        srcw[:, p * DT * P + u * h : p * DT * P + (u + 1) * h],
                    )
                return t

            # PE warm-up: junk transposes during the initial input DMA so
            # the tensor engine is at full clock when real matmuls arrive
            for _ in range(25):
                wpt = ps_ps.tile([P, SB], F16, tag="ps", name="warm")
                for u in range(SB // P):
                    nc.tensor.transpose(
                        wpt[:, u * P : (u + 1) * P], ident[:], ident[:]
                    )

            wtiles = {}  # (p, nm) -> wgt tile
            wtiles[(0, "q")] = load_wgt(wq, 0, "wq0")
            wtiles[(0, "k")] = load_wgt(wk, 0, "wk0")
            wtiles[(0, "v")] = load_wgt(wv, 0, "wv0")
            # j-sliced xT load: the first half (s-blocks 0-1) lands first so
            # pair-0's early projection chains start well before the full
            # 4MB arrives
            xt = [
                xt_pool.tile([P, S], F16, tag="xt", name=f"xt{d}")
                for d in range(DT)
            ]
            for jj in range(0, NSB, 2):
                for d in range(DT):
                    nc.sync.dma_start(
                        xt[d][:, jj * SB : (jj + 2) * SB],
                        xT[d * P : (d + 1) * P, jj * SB : (jj + 2) * SB],
                    )

            # persistent V' tiles, one per in-flight pair; layout
            # [t, g*2048 + tg*128 + (0:64 V | 64:128 ones)]; the ones
            # halves are written once and survive slot reuse.
            vp_slots = []
            for s in range(2):
                v = vp_pool.tile([P, 2 * TT * P], F16, tag=f"vp{s}", bufs=1,
                                 name=f"vp{s}")
                nc.vector.memset(
                    v[:].rearrange("r (g tg c) -> r g tg c", g=2, tg=TT)[
                        :, :, :, DK:P
                    ],
                    1.0,
                )
                vp_slots.append(v)

            qt_tiles = {}
            kt_tiles = {}
            ot_tiles = []

            # ---- work units (filler between attention chunks) ----
            filler = deque()  # (key, closure)

            reserve = [0]  # units held back for the epilogue

            def drain(n):
                if len(filler) > 12:
                    n += 1
                while n > 0 and len(filler) > reserve[0]:
                    filler.popleft()[1]()
                    n -= 1

            def drain_until(key):
                while any(k <= key for k, _ in filler):
                    filler.popleft()[1]()

            def push_proj_unit(p, nm, j, dest_ap, bias_t):
                def unit():
                    ps = ps_ps.tile([P, SB], F32, tag="ps", name="ps_p")
                    w = wtiles[(p, nm)]
                    for d in range(DT):
                        nc.tensor.matmul(
                            ps[:],
                            w[:, d * P : (d + 1) * P],
                            xt[d][:, j * SB : (j + 1) * SB],
                            start=(d == 0),
                            stop=(d == DT - 1),
                        )
                    nc.vector.tensor_scalar_add(
                        dest_ap, ps[:], bias_t[:, p : p + 1],
                    )

                filler.append(((p, j), unit))

            def push_vtrans_unit(p, j, vst):
                def unit():
                    pt = ps_ps.tile([P, SB], F16, tag="ps", name="pt")
                    for u in range(SB // P):
                        nc.tensor.transpose(
                            pt[:, u * P : (u + 1) * P],
                            vst[:, u * P : (u + 1) * P],
                            ident[:],
                        )
                    # scatter [t, (u, g, c)] -> vp[t, (g, 4j+u, c)]
                    vpt = vp_slots[p % 2]
                    dst = vpt[:].rearrange(
                        "r (g tg c) -> r g tg c", g=2, tg=TT
                    )[:, :, 4 * j : 4 * j + 4, 0:DK]
                    src = pt[:].rearrange(
                        "r (u g c) -> r g u c", u=SB // P, g=2
                    )
                    nc.vector.tensor_copy(dst, src)

                filler.append(((p, j), unit))

            def push_pair_units(p):
                qt = qt_pool.tile([P, S], F16, tag="qt", name=f"qt{p}")
                kt = kt_pool.tile([P, S], F16, tag="kt", name=f"kt{p}")
                qt_tiles[p] = qt
                kt_tiles[p] = kt
                if p > 0:
                    for nm, srcw in (("q", wq), ("k", wk), ("v", wv)):
                        wtiles[(p, nm)] = load_wgt(srcw, p, f"w{nm}{p}")
                # vt-j is pushed one unit after its v-j so the transposes
                # don't immediately wait on the DVE bias-copy of vst
                pend_vt = None
                for j in range(NSB):
                    push_proj_unit(p, "q", j, qt[:, j * SB : (j + 1) * SB],
                                   bq_t)
                    if pend_vt is not None:
                        push_vtrans_unit(p, pend_vt[0], pend_vt[1])
                    push_proj_unit(p, "k", j, kt[:, j * SB : (j + 1) * SB],
                                   bk_t)
                    vst = vst_pool.tile([P, SB], F16, tag="vst", name="vst")
                    push_proj_unit(p, "v", j, vst[:], bv_t)
                    pend_vt = (j, vst)
                push_vtrans_unit(p, pend_vt[0], pend_vt[1])

            def push_oproj_unit(m, j, wo_tiles):
                def unit():
                    ps = ps_ps.tile([P, SB], F32, tag="ps", name="ps_o")
                    for p in range(NPAIR):
                        nc.tensor.matmul(
                            ps[:],
                            wo_tiles[p][:, m * P : (m + 1) * P],
                            ot_tiles[p][:, j * SB : (j + 1) * SB],
                            start=(p == 0),
                            stop=(p == NPAIR - 1),
                        )
                    st = ost_pool.tile([P, SB], F16, tag="ost", name="ost")
                    nc.vector.tensor_copy(st[:], ps[:])
                    nc.sync.dma_start(
                        out[m * P : (m + 1) * P, j * SB : (j + 1) * SB],
                        st[:],
                    )

                filler.append(((NPAIR, j, m), unit))

            # ---- main schedule ----
            # close(j): final AV flush + softmax normalization for the most
            # recently finished j-block (possibly of the previous pair).
            # Deferred until after the next score batch so every j/pair
            # boundary is filled with PE work while the pa-release copy runs.
            close_state = [None]  # (p, j, pa, pend, vpt, ot)

            def issue_av(pa, nt, vpt, item):
                i, wt, c0 = item
                for g in range(2):
                    nc.tensor.matmul(
                        pa[:, g * SB + c0 : (g + 1) * SB],
                        vpt[:, g * TT * P + i * P : g * TT * P + (i + 1) * P],
                        wt[:, g * SB + c0 : (g + 1) * SB],
                        start=(i == 0),
                        stop=(i == nt - 1),
                    )

            def close_j():
                if close_state[0] is None:
                    return
                cp, j, pa, pend_items, vpt, ot = close_state[0]
                close_state[0] = None
                nt = 4 * j + 4
                for item in pend_items:
                    issue_av(pa, nt, vpt, item)
                pac = den_pool.tile([P, 2 * SB], F32, tag="den", name="pac")
                nc.scalar.copy(pac[:], pa[:])
                den = den_pool.tile([DK, 2 * SB], F32, tag="dend",
                                    name="den")
                nc.vector.tensor_copy(den[:], pac[DK:P, :])
                rcs = rcs_pool.tile([DK, 2 * SB], F32, tag="rcs", name="rcs")
                nc.vector.reciprocal_approx_fast(rcs[:], den[:])
                for g in range(2):
                    nc.vector.tensor_tensor(
                        ot[g * DK : (g + 1) * DK, j * SB : (j + 1) * SB],
                        pac[0:DK, g * SB : (g + 1) * SB],
                        rcs[:, g * SB : (g + 1) * SB],
                        MUL,
                    )
                if cp == NPAIR - 1:
                    for m in range(DT):
                        push_oproj_unit(m, j, wo_tiles)
                drain(1)

            push_pair_units(0)
            wo_tiles = []
            for p in range(NPAIR):
                qt = qt_tiles[p]
                kt = kt_tiles[p]
                vpt = vp_slots[p % 2]
                ot = ot_pool.tile([P, S], F16, tag="ot", name=f"ot{p}")
                ot_tiles.append(ot)
                if p + 1 < NPAIR:
                    push_pair_units(p + 1)
                else:
                    # wo loads land during pair-3 attention; hold back a few
                    # filler units so the epilogue has ready PE work to
                    # overlap the final rescale chain
                    reserve[0] = 6
                    for pp in range(NPAIR):
                        t = wo_pool.tile([P, D], F16, tag="wo", name=f"wo{pp}")
                        nc.sync.dma_start(t[:], wo_t[pp * P : (pp + 1) * P, :])
                        wo_tiles.append(t)

                for j in range(NSB):
                    drain_until((p, j))
                    nt = 4 * j + 4
                    pa = None
                    pend = []

                    # two chunks per batch: the four 64-contraction score
                    # matmuls run in one 64x128 row-tiled PE stretch (heads
                    # at tile rows 0/64 execute concurrently), paying the
                    # 128<->64 mode-switch drain once per batch.
                    for i2 in range(0, nt, 2):
                        batch = []
                        for i in (i2, i2 + 1):
                            r = i - 4 * j
                            c0 = P * max(r, 0)
                            sc = ps_sc.tile([P, 2 * SB], F32, tag="sc",
                                            name="sc")
                            for g in range(2):
                                nc.tensor.matmul(
                                    sc[:, g * SB + c0 : (g + 1) * SB],
                                    kt[g * DK : (g + 1) * DK,
                                       i * P : (i + 1) * P],
                                    qt[g * DK : (g + 1) * DK,
                                       j * SB + c0 : (j + 1) * SB],
                                    start=True,
                                    stop=True,
                                )
                            batch.append((i, sc, c0))
                        new_pend = []
                        for i, sc, c0 in batch:
                            r = i - 4 * j
                            wt = wt_pool.tile([P, 2 * SB], F16, tag="wt",
                                              name="wt")
                            if r >= 0:
                                scv = sc[:].rearrange(
                                    "r (g w) -> r g w", g=2
                                )[:, :, c0:]
                                wtv = wt[:].rearrange(
                                    "r (g w) -> r g w", g=2
                                )[:, :, c0:]
                            else:
                                scv = sc[:]
                                wtv = wt[:]
                            nc.scalar.activation(wtv, scv, AF.Exp,
                                                 scale=0.125)
                            if r >= 0:
                                wmv = wt[:].rearrange(
                                    "r (g w) -> r g w", g=2
                                )[:, :, c0 : c0 + P]
                                nc.vector.tensor_tensor(
                                    wmv,
                                    wmv,
                                    mask2[:].rearrange(
                                        "r (g w) -> r g w", g=2
                                    ),
                                    MUL,
                                )
                            new_pend.append((i, wt, c0))
                        if i2 == 0:
                            # previous j's final AVs + rescale run behind
                            # this batch's scores/exps
                            close_j()
                            pa = ps_pa.tile([P, 2 * SB], F32, tag="pa",
                                            name="pa")
                        else:
                            drain(1)
                            for item in pend:
                                issue_av(pa, nt, vpt, item)
                        pend = new_pend
                        drain(1)
                    close_state[0] = (p, j, pa, pend, vpt, ot)

            # close the final j-block of the last pair
            close_j()

            # epilogue: drain remaining output-projection units
            while filler:
                filler.popleft()[1]()

    nc.compile()
    return nc


_NC_CACHE = None


def _get_nc():
    global _NC_CACHE
    if _NC_CACHE is None:
        _NC_CACHE = build_nc()
    return _NC_CACHE


def _core_inputs(x, Wq, bq, Wk, bk, Wv, bv, Wo, c):
    b, g = c // 2, c % 2
    heads = range(g * HPC, (g + 1) * HPC)
    xT = np.ascontiguousarray(x[b].T, dtype=np.float16)
    def warr(W):
        w = np.concatenate([W[h] for h in heads], axis=1)  # [D, 512]
        # [r, p*1024 + d*128 + c] = w[d*128 + r, p*128 + c]
        blocks = [
            w[:, p * P : (p + 1) * P]
            .reshape(DT, P, P)
            .transpose(1, 0, 2)
            .reshape(P, DT * P)
            for p in range(NPAIR)
        ]
        return np.ascontiguousarray(
            np.concatenate(blocks, axis=1), dtype=np.float16
        )

    wq_c = warr(Wq)
    wk_c = warr(Wk)
    wv_c = warr(Wv)
    bq_c = np.ascontiguousarray(
        np.concatenate([bq[h] for h in heads]).reshape(NPAIR, P).T, dtype=np.float32
    )
    bk_c = np.ascontiguousarray(
        np.concatenate([bk[h] for h in heads]).reshape(NPAIR, P).T, dtype=np.float32
    )
    bv_c = np.ascontiguousarray(
        np.concatenate([bv[h] for h in heads]).reshape(NPAIR, P).T, dtype=np.float32
    )
    wo_c = np.ascontiguousarray(
        Wo[:, g * HPC * DK : (g + 1) * HPC * DK].T, dtype=np.float16
    )
    return {
        "xT": xT,
        "wq": wq_c,
        "wk": wk_c,
        "wv": wv_c,
        "wo_t": wo_c,
        "bq": bq_c,
        "bk": bk_c,
        "bv": bv_c,
    }


def kernel(x, Wq, bq, Wk, bk, Wv, bv, Wo, bo, _trace=False, _tmpdir=None):
    x = np.asarray(x, dtype=np.float32)
    nc = _get_nc()
    in_maps = [
        _core_inputs(x, Wq, bq, Wk, bk, Wv, bv, Wo, c) for c in range(NCORES)
    ]
    kw = {}
    if _trace:
        kw = dict(trace=True, tmpdir=_tmpdir)
    res = bass_utils.run_bass_kernel_spmd(
        nc, in_maps, core_ids=list(range(NCORES)), **kw
    )
    bo = np.asarray(bo, dtype=np.float32)
    out = np.empty((B, S, D), dtype=np.float32)
    for b in range(B):
        part = res.results[2 * b]["out_part"].astype(np.float32) + res.results[
            2 * b + 1
        ]["out_part"].astype(np.float32)
        out[b] = part.T + bo
    if _trace:
        kernel._last_results = res
    return out


# revision 40
# speedup vs baseline: 1.0612x; 1.0039x over previous
"""Trainium2 Bass kernel for 16-head causal MultiHeadAttention.

Problem: x [4, 2048, 1024], 16 heads of dim 64, causal softmax attention,
output projection Wo [1024, 1024] + bo.

Sharding over 8 NeuronCores: core c handles batch b = c // 2 and head-group
g = c % 2 (8 heads each).  Each core computes its 8 heads' Q/K/V projections,
causal attention, and a partial output projection against its row-slice of
Wo.  The two cores of a batch return partial [D, S] outputs that the host
sums, transposes, and biases.

Schedule: the attention inner loop is software-pipelined per chunk (one
128-row t-tile, both heads of a pair side by side): scores matmuls ->
merged causally-trimmed exp (ACT) -> lag-1 AV matmuls, with a filler queue
of projection / V-transpose / output-projection work units drained between
chunks so the PE never idles while ACT runs exp.  Work units are j-ordered
so pair p's attention at s-block j only needs units (p, <=j), letting
pair-0 projections interleave with pair-0 attention (tiny prologue).

Score matmuls contract over dk=64, so the two heads' matmuls run
concurrently in 64x128 row-tiled PE mode (tile positions inferred from
the operands' base partitions); they are issued in batches of two chunks
(4 matmuls) per 64-mode stretch so the 128<->64 mode-switch drain is paid
once per batch.  A junk-transpose warm-up keeps the PE clock ramped
through the initial input-DMA wait; each j-block's softmax normalization
releases the single pa accumulator via one fast ACT copy, with
reciprocal+rescale running off the SBUF copy (reciprocal_approx_fast must
read partition offset 0).  Weights are pre-rearranged host-side so each
pair's projection weights load as plain 2KB-per-partition-line DMAs, and
the partial output is returned as f16 to halve the output-DMA tail.

PSUM (8 banks): sc [128,1024]x2 (scores, both heads) + pa [128,1024]x1
(AV accum + softmax denominators via ones-columns of V') + ps [128,512]x2
(projection groups / V-transposes / output projection / warm-up).
"""

import sys
from collections import deque

for _p in ("/opt/trn_rl_repo", "/root/.axon_site/_ro/trn_rl_repo"):
    if _p not in sys.path:
        sys.path.insert(0, _p)

import numpy as np

import concourse.bacc as bacc
import concourse.mybir as mybir
from concourse import bass_utils
from concourse.masks import make_identity, make_upper_triangular
from concourse.tile import TileContext

P = 128
S = 2048  # sequence length
D = 1024  # hidden size
H = 16  # total heads
DK = 64  # head dim
B = 4  # batch
NCORES = 8
HPC = 8  # heads per core
NPAIR = HPC // 2  # head pairs per core
SB = 512  # s-block width
NSB = S // SB  # 4
TT = S // P  # 16 t-tiles
DT = D // P  # 8 d-tiles

F32 = mybir.dt.float32
F16 = mybir.dt.float16
AF = mybir.ActivationFunctionType
MUL = mybir.AluOpType.mult


def build_nc(debug=False):
    nc = bacc.Bacc()
    xT = nc.dram_tensor("xT", [D, S], F16, kind="ExternalInput")
    # weights pre-rearranged host-side: [r, p*1024 + d*128 + c] =
    # W[d*128 + r, p*128 + c], so each pair loads as one plain
    # [128, 1024] DMA with 2KB partition lines
    wq = nc.dram_tensor("wq", [P, NPAIR * DT * P], F16, kind="ExternalInput")
    wk = nc.dram_tensor("wk", [P, NPAIR * DT * P], F16, kind="ExternalInput")
    wv = nc.dram_tensor("wv", [P, NPAIR * DT * P], F16, kind="ExternalInput")
    wo_t = nc.dram_tensor("wo_t", [HPC * DK, D], F16, kind="ExternalInput")
    bq = nc.dram_tensor("bq", [P, NPAIR], F32, kind="ExternalInput")
    bk = nc.dram_tensor("bk", [P, NPAIR], F32, kind="ExternalInput")
    bv = nc.dram_tensor("bv", [P, NPAIR], F32, kind="ExternalInput")
    out = nc.dram_tensor("out_part", [D, S], F16, kind="ExternalOutput")

    with TileContext(nc) as tc:
        from contextlib import ExitStack

        with ExitStack() as ctx:
            pool = lambda *a, **k: ctx.enter_context(tc.tile_pool(*a, **k))
            xt_pool = pool(name="xt", bufs=DT)
            wgt_pool = pool(name="wgt", bufs=6)
            wo_pool = pool(name="wo", bufs=NPAIR)
            qt_pool = pool(name="qt", bufs=2)
            kt_pool = pool(name="kt", bufs=2)
            vp_pool = pool(name="vp", bufs=2)
            vst_pool = pool(name="vst", bufs=2)
            wt_pool = pool(name="wt", bufs=6)
            ot_pool = pool(name="ot", bufs=NPAIR)
            den_pool = pool(name="den", bufs=2)
            rcs_pool = pool(name="rcs", bufs=2)
            ost_pool = pool(name="ost", bufs=3)
            const_pool = pool(name="const", bufs=1)
            ps_sc = pool(name="ps_sc", bufs=2, space="PSUM")
            ps_pa = pool(name="ps_pa", bufs=1, space="PSUM")
            ps_ps = pool(name="ps_ps", bufs=2, space="PSUM")

            # --- constants ---
            ident = const_pool.tile([P, P], F16)
            make_identity(nc, ident[:])
            # mask2[r, g*128+c] = 1 if c >= r else 0, for both head groups
            mask2 = const_pool.tile([P, 2 * P], F16)
            make_upper_triangular(nc, mask2[:, 0:P], val=1.0, diag=True)
            make_upper_triangular(nc, mask2[:, P : 2 * P], val=1.0, diag=True)
            bq_t = const_pool.tile([P, NPAIR], F32)
            nc.sync.dma_start(bq_t[:], bq[:])
            bk_t = const_pool.tile([P, NPAIR], F32)
            nc.sync.dma_start(bk_t[:], bk[:])
            bv_t = const_pool.tile([P, NPAIR], F32)
            nc.sync.dma_start(bv_t[:], bv[:])

            # --- resident inputs ---
            def load_wgt(srcw, p, name, splits=2):
                t = wgt_pool.tile([P, DT * P], F16, tag="wgt", name=name)
                h = DT * P // splits
                for u in range(splits):
                    nc.sync.dma_start(
                        t[:, u * h : (u + 1) * h],
                        srcw[:, p * DT * P + u * h : p * DT * P + (u + 1) * h],
                    )
                return t

            # PE warm-up: junk transposes during the initial input DMA so
            # the tensor engine is at full clock when real matmuls arrive
            for _ in range(25):
                wpt = ps_ps.tile([P, SB], F16, tag="ps", name="warm")
                for u in range(SB // P):
                    nc.tensor.transpose(
                        wpt[:, u * P : (u + 1) * P], ident[:], ident[:]
                    )

            wtiles = {}  # (p, nm) -> wgt tile
            wtiles[(0, "q")] = load_wgt(wq, 0, "wq0")
            wtiles[(0, "k")] = load_wgt(wk, 0, "wk0")
            wtiles[(0, "v")] = load_wgt(wv, 0, "wv0")
            # j-sliced xT load: the first half (s-blocks 0-1) lands first so
            # pair-0's early projection chains start well before the full
            # 4MB arrives
            xt = [
                xt_pool.tile([P, S], F16, tag="xt", name=f"xt{d}")
                for d in range(DT)
            ]
            for jj in range(0, NSB, 2):
                for d in range(DT):
                    nc.sync.dma_start(
                        xt[d][:, jj * SB : (jj + 2) * SB],
                        xT[d * P : (d + 1) * P, jj * SB : (jj + 2) * SB],
                    )

            # persistent V' tiles, one per in-flight pair; layout
            # [t, g*2048 + tg*128 + (0:64 V | 64:128 ones)]; the ones
            # halves are written once and survive slot reuse.
            vp_slots = []
            for s in range(2):
                v = vp_pool.tile([P, 2 * TT * P], F16, tag=f"vp{s}", bufs=1,
                                 name=f"vp{s}")
                nc.vector.memset(
                    v[:].rearrange("r (g tg c) -> r g tg c", g=2, tg=TT)[
                        :, :, :, DK:P
                    ],
                    1.0,
                )
                vp_slots.append(v)

            qt_tiles = {}
            kt_tiles = {}
            ot_tiles = []

            # ---- work units (filler between attention chunks) ----
            filler = deque()  # (key, closure)

            reserve = [0]  # units held back for the epilogue

            def drain(n):
                if len(filler) > 12:
                    n += 1
                while n > 0 and len(filler) > reserve[0]:
                    filler.popleft()[1]()
                    n -= 1

            def drain_until(key):
                while any(k <= key for k, _ in filler):
                    filler.popleft()[1]()

            def push_proj_unit(p, nm, j, dest_ap, bias_t):
                def unit():
                    ps = ps_ps.tile([P, SB], F32, tag="ps", name="ps_p")
                    w = wtiles[(p, nm)]
                    for d in range(DT):
                        nc.tensor.matmul(
                            ps[:],
                            w[:, d * P : (d + 1) * P],
                            xt[d][:, j * SB : (j + 1) * SB],
                            start=(d == 0),
                            stop=(d == DT - 1),
                        )
                    nc.vector.tensor_scalar_add(
                        dest_ap, ps[:], bias_t[:, p : p + 1],
                    )

                filler.append(((p, j), unit))

            def push_vtrans_unit(p, j, vst):
                def unit():
                    pt = ps_ps.tile([P, SB], F16, tag="ps", name="pt")
                    for u in range(SB // P):
                        nc.tensor.transpose(
                            pt[:, u * P : (u + 1) * P],
                            vst[:, u * P : (u + 1) * P],
                            ident[:],
                        )
                    # scatter [t, (u, g, c)] -> vp[t, (g, 4j+u, c)]
                    vpt = vp_slots[p % 2]
                    dst = vpt[:].rearrange(
                        "r (g tg c) -> r g tg c", g=2, tg=TT
                    )[:, :, 4 * j : 4 * j + 4, 0:DK]
                    src = pt[:].rearrange(
                        "r (u g c) -> r g u c", u=SB // P, g=2
                    )
                    nc.vector.tensor_copy(dst, src)

                filler.append(((p, j), unit))

            def push_pair_units(p):
                qt = qt_pool.tile([P, S], F16, tag="qt", name=f"qt{p}")
                kt = kt_pool.tile([P, S], F16, tag="kt", name=f"kt{p}")
                qt_tiles[p] = qt
                kt_tiles[p] = kt
                if p > 0:
                    for nm, srcw in (("q", wq), ("k", wk), ("v", wv)):
                        wtiles[(p, nm)] = load_wgt(srcw, p, f"w{nm}{p}")
                # vt-j is pushed one unit after its v-j so the transposes
                # don't immediately wait on the DVE bias-copy of vst
                pend_vt = None
                for j in range(NSB):
                    push_proj_unit(p, "q", j, qt[:, j * SB : (j + 1) * SB],
                                   bq_t)
                    if pend_vt is not None:
                        push_vtrans_unit(p, pend_vt[0], pend_vt[1])
                    push_proj_unit(p, "k", j, kt[:, j * SB : (j + 1) * SB],
                                   bk_t)
                    vst = vst_pool.tile([P, SB], F16, tag="vst", name="vst")
                    push_proj_unit(p, "v", j, vst[:], bv_t)
                    pend_vt = (j, vst)
                push_vtrans_unit(p, pend_vt[0], pend_vt[1])

            def push_oproj_unit(m, j, wo_tiles):
                def unit():
                    ps = ps_ps.tile([P, SB], F32, tag="ps", name="ps_o")
                    for p in range(NPAIR):
                        nc.tensor.matmul(
                            ps[:],
                            wo_tiles[p][:, m * P : (m + 1) * P],
                            ot_tiles[p][:, j * SB : (j + 1) * SB],
                            start=(p == 0),
                            stop=(p == NPAIR - 1),
                        )
                    st = ost_pool.tile([P, SB], F16, tag="ost", name="ost")
                    nc.vector.tensor_copy(st[:], ps[:])
                    nc.sync.dma_start(
                        out[m * P : (m + 1) * P, j * SB : (j + 1) * SB],
                        st[:],
                    )

                filler.append(((NPAIR, j, m), unit))

            # ---- main schedule ----
            # close(j): final AV flush + softmax normalization for the most
            # recently finished j-block (possibly of the previous pair).
            # Deferred until after the next score batch so every j/pair
            # boundary is filled with PE work while the pa-release copy runs.
            close_state = [None]  # (p, j, pa, pend, vpt, ot)

            def issue_av(pa, nt, vpt, item):
                i, wt, c0 = item
                for g in range(2):
                    nc.tensor.matmul(
                        pa[:, g * SB + c0 : (g + 1) * SB],
                        vpt[:, g * TT * P + i * P : g * TT * P + (i + 1) * P],
                        wt[:, g * SB + c0 : (g + 1) * SB],
                        start=(i == 0),
                        stop=(i == nt - 1),
                    )

            def close_j():
                if close_state[0] is None:
                    return
                cp, j, pa, pend_items, vpt, ot = close_state[0]
                close_state[0] = None
                nt = 4 * j + 4
                for item in pend_items:
                    issue_av(pa, nt, vpt, item)
                pac = den_pool.tile([P, 2 * SB], F32, tag="den", name="pac")
                nc.scalar.copy(pac[:], pa[:])
                den = den_pool.tile([DK, 2 * SB], F32, tag="dend",
                                    name="den")
                nc.vector.tensor_copy(den[:], pac[DK:P, :])
                rcs = rcs_pool.tile([DK, 2 * SB], F32, tag="rcs", name="rcs")
                nc.vector.reciprocal_approx_fast(rcs[:], den[:])
                for g in range(2):
                    nc.vector.tensor_tensor(
                        ot[g * DK : (g + 1) * DK, j * SB : (j + 1) * SB],
                        pac[0:DK, g * SB : (g + 1) * SB],
                        rcs[:, g * SB : (g + 1) * SB],
                        MUL,
                    )
                if cp == NPAIR - 1:
                    for m in range(DT):
                        push_oproj_unit(m, j, wo_tiles)
                drain(1)

            push_pair_units(0)
            wo_tiles = []
            for p in range(NPAIR):
                qt = qt_tiles[p]
                kt = kt_tiles[p]
                vpt = vp_slots[p % 2]
                ot = ot_pool.tile([P, S], F16, tag="ot", name=f"ot{p}")
                ot_tiles.append(ot)
                if p + 1 < NPAIR:
                    push_pair_units(p + 1)
                else:
                    # wo loads land during pair-3 attention; hold back a few
                    # filler units so the epilogue has ready PE work to
                    # overlap the final rescale chain
                    reserve[0] = 9
                    for pp in range(NPAIR):
                        t = wo_pool.tile([P, D], F16, tag="wo", name=f"wo{pp}")
                        nc.sync.dma_start(t[:], wo_t[pp * P : (pp + 1) * P, :])
                        wo_tiles.append(t)

                for j in range(NSB):
                    drain_until((p, j))
                    nt = 4 * j + 4
                    pa = None
                    pend = []

                    # two chunks per batch: the four 64-contraction score
                    # matmuls run in one 64x128 row-tiled PE stretch (heads
                    # at tile rows 0/64 execute concurrently), paying the
                    # 128<->64 mode-switch drain once per batch.
                    for i2 in range(0, nt, 2):
                        batch = []
                        for i in (i2, i2 + 1):
                            r = i - 4 * j
                            c0 = P * max(r, 0)
                            sc = ps_sc.tile([P, 2 * SB], F32, tag="sc",
                                            name="sc")
                            for g in range(2):
                                nc.tensor.matmul(
                                    sc[:, g * SB + c0 : (g + 1) * SB],
                                    kt[g * DK : (g + 1) * DK,
                                       i * P : (i + 1) * P],
                                    qt[g * DK : (g + 1) * DK,
                                       j * SB + c0 : (j + 1) * SB],
                                    start=True,
                                    stop=True,
                                )
                            batch.append((i, sc, c0))
                        new_pend = []
                        for i, sc, c0 in batch:
                            r = i - 4 * j
                            wt = wt_pool.tile([P, 2 * SB], F16, tag="wt",
                                              name="wt")
                            if r >= 0:
                                scv = sc[:].rearrange(
                                    "r (g w) -> r g w", g=2
                                )[:, :, c0:]
                                wtv = wt[:].rearrange(
                                    "r (g w) -> r g w", g=2
                                )[:, :, c0:]
                            else:
                                scv = sc[:]
                                wtv = wt[:]
                            nc.scalar.activation(wtv, scv, AF.Exp,
                                                 scale=0.125)
                            if r >= 0:
                                wmv = wt[:].rearrange(
                                    "r (g w) -> r g w", g=2
                                )[:, :, c0 : c0 + P]
                                nc.vector.tensor_tensor(
                                    wmv,
                                    wmv,
                                    mask2[:].rearrange(
                                        "r (g w) -> r g w", g=2
                                    ),
                                    MUL,
                                )
                            new_pend.append((i, wt, c0))
                        if i2 == 0:
                            # previous j's final AVs + rescale run behind
                            # this batch's scores/exps
                            close_j()
                            pa = ps_pa.tile([P, 2 * SB], F32, tag="pa",
                                            name="pa")
                        else:
                            drain(1)
                            for item in pend:
                                issue_av(pa, nt, vpt, item)
                        pend = new_pend
                        drain(1)
                    close_state[0] = (p, j, pa, pend, vpt, ot)

            # close the final j-block of the last pair
            close_j()

            # epilogue: drain remaining output-projection units
            while filler:
                filler.popleft()[1]()

    nc.compile()
    return nc


_NC_CACHE = None


def _get_nc():
    global _NC_CACHE
    if _NC_CACHE is None:
        _NC_CACHE = build_nc()
    return _NC_CACHE


def _core_inputs(x, Wq, bq, Wk, bk, Wv, bv, Wo, c):
    b, g = c // 2, c % 2
    heads = range(g * HPC, (g + 1) * HPC)
    xT = np.ascontiguousarray(x[b].T, dtype=np.float16)
    def warr(W):
        w = np.concatenate([W[h] for h in heads], axis=1)  # [D, 512]
        # [r, p*1024 + d*128 + c] = w[d*128 + r, p*128 + c]
        blocks = [
            w[:, p * P : (p + 1) * P]
            .reshape(DT, P, P)
            .transpose(1, 0, 2)
            .reshape(P, DT * P)
            for p in range(NPAIR)
        ]
        return np.ascontiguousarray(
            np.concatenate(blocks, axis=1), dtype=np.float16
        )

    wq_c = warr(Wq)
    wk_c = warr(Wk)
    wv_c = warr(Wv)
    bq_c = np.ascontiguousarray(
        np.concatenate([bq[h] for h in heads]).reshape(NPAIR, P).T, dtype=np.float32
    )
    bk_c = np.ascontiguousarray(
        np.concatenate([bk[h] for h in heads]).reshape(NPAIR, P).T, dtype=np.float32
    )
    bv_c = np.ascontiguousarray(
        np.concatenate([bv[h] for h in heads]).reshape(NPAIR, P).T, dtype=np.float32
    )
    wo_c = np.ascontiguousarray(
        Wo[:, g * HPC * DK : (g + 1) * HPC * DK].T, dtype=np.float16
    )
    return {
        "xT": xT,
        "wq": wq_c,
        "wk": wk_c,
        "wv": wv_c,
        "wo_t": wo_c,
        "bq": bq_c,
        "bk": bk_c,
        "bv": bv_c,
    }


def kernel(x, Wq, bq, Wk, bk, Wv, bv, Wo, bo, _trace=False, _tmpdir=None):
    x = np.asarray(x, dtype=np.float32)
    nc = _get_nc()
    in_maps = [
        _core_inputs(x, Wq, bq, Wk, bk, Wv, bv, Wo, c) for c in range(NCORES)
    ]
    kw = {}
    if _trace:
        kw = dict(trace=True, tmpdir=_tmpdir)
    res = bass_utils.run_bass_kernel_spmd(
        nc, in_maps, core_ids=list(range(NCORES)), **kw
    )
    bo = np.asarray(bo, dtype=np.float32)
    out = np.empty((B, S, D), dtype=np.float32)
    for b in range(B):
        part = res.results[2 * b]["out_part"].astype(np.float32) + res.results[
            2 * b + 1
        ]["out_part"].astype(np.float32)
        out[b] = part.T + bo
    if _trace:
        kernel._last_results = res
    return out
